# revision 45
# baseline (speedup 1.0000x reference)
"""MoE kernel for Trainium2 (8 NeuronCores, expert-parallel sparse dispatch).

Problem (hardcoded): B=2, S=2048, D=1024, E=8 experts, F=4096, top-K=2.
out = x + sum_{k in top2} w_k * (gelu(x @ w1[e_k] + b1[e_k]) @ w2[e_k] + b2[e_k])

Strategy: the router (0.01% of FLOPs) runs on host; tokens are dispatched
expert-parallel to the 8 cores (core i gets expert i's routed tokens) with
capacity 1024 = N*TOPK/E; overflow tokens of hot experts (78 for the seed-0
routing) are computed exactly on host, so every core carries the mean load.
Each core runs a dense FFN over its 1024 tokens in fp8e4 (e4m3) with
DoubleRow matmuls - the PE packs 2 fp8 weights/cell, virtualizing the array
to 256 contraction rows at 2 MACs/cell/cycle, 2x the bf16/f32r rate. All
operands live in transposed layout ([dim, token]); weights are pre-scaled
x1024 on host (else they land in the fp8 subnormal range) and dequantized
for free via the activation-engine scale. x / gelu(h) quantize unscaled.
Head DMAs spread across the sync/act/gpsimd queues (~600ns/trigger,
~90GB/s/queue) so real matmuls start ~12us in; 88 dummy matmuls keep the
PE busy through that window so the HAM clock gate never throttles the real
stream. Host scatter-adds the weighted expert outputs and the residual.

Measured on the seed-0 instance: HW exec 134.3 us (max core, 3-run stable)
vs 285 us for the f32r baseline; rel max err 1.767e-2 (gate 2e-2),
dominated by e4m3 quantization noise (~2.7% per tensor, x/w1/g/w2 equal).
Breakdown: ~7us SPMD init, real MMs 11.8->128us (512 MMs at 213ns = fp8
DoubleRow peak, ~3us idle), ~6.3us tail (final ACT chain + store + drains).
PRECISION selects "fp8" (default), "f32r" (~285 us, 7e-5), or "bf16".
"""

import os as _os

import numpy as np
import ml_dtypes

try:
    from scipy.special import erf as _erf
except ImportError:                     # exact erf fallback, vectorized
    import math as _math
    _erf = np.vectorize(_math.erf, otypes=[np.float64])

B, S, D, E, F, TOPK = 2, 2048, 1024, 8, 4096, 2
N = B * S           # 4096 tokens
P = 128             # partitions
ND = D // P         # 8 chunks of the model dim
NF = F // P         # 32 chunks of the hidden dim
NT = 512            # token tile (matmul free dim; one PSUM bank of fp32)

BF16 = ml_dtypes.bfloat16

_ACT_FUNC = "Gelu"  # overridden to "Identity" by the CoreSim check only

_cache = {}


def _tile_plan(C):
    """Split C tokens into matmul free-dim tiles (multiples of 128)."""
    tiles = [NT] * (C // NT)
    if C % NT:
        tiles.append(C % NT)
    return tiles


# Max tokens processed per weight-stream pass (PSUM: <=4 slices of 512,
# and SBUF must hold x + g for the whole super-tile).
SUPER = 1536
SUPER_F32R = 1152


def _slice_plan_256(C):
    """Split C into slices of multiple-of-32 sizes, each in [256, 512]
    (float32r matmul runs 4x slower below a 256-wide moving operand)."""
    if C <= 512:
        return [C]
    n = -(-C // 512)
    base = C // n // 32 * 32
    sizes = [base] * n
    rem = C - base * n
    i = 0
    while rem > 0:
        add = min(32, rem)
        sizes[i % n] += add
        rem -= add
        i += 1
    return sizes


def _build(C):
    import concourse.mybir as mybir
    import concourse.tile as tile
    from concourse import bacc

    dt = mybir.dt
    AF = mybir.ActivationFunctionType

    nc = bacc.Bacc("TRN2", target_bir_lowering=False, debug=False)
    xt = nc.dram_tensor("xt", (P, ND, C), dt.bfloat16, kind="ExternalInput")
    w1 = nc.dram_tensor("w1", (NF, P, ND, P), dt.bfloat16, kind="ExternalInput")
    b1 = nc.dram_tensor("b1", (P, NF), dt.float32, kind="ExternalInput")
    w2 = nc.dram_tensor("w2", (ND, P, NF, P), dt.bfloat16, kind="ExternalInput")
    b2 = nc.dram_tensor("b2", (P, ND), dt.float32, kind="ExternalInput")
    yt = nc.dram_tensor("yt", (P, ND, C), dt.float32, kind="ExternalOutput")

    with tile.TileContext(nc) as tc:
        with (
            tc.tile_pool(name="consts", bufs=1) as consts,
            tc.tile_pool(name="xp", bufs=1) as xp,
            tc.tile_pool(name="w1p", bufs=4) as w1p,
            tc.tile_pool(name="w2p", bufs=2) as w2p,
            tc.tile_pool(name="gp", bufs=1) as gp,
            tc.tile_pool(name="yp", bufs=2) as yp,
            tc.tile_pool(name="psum", bufs=2, space="PSUM") as psum,
        ):
            b1_sb = consts.tile([P, NF], dt.float32)
            nc.sync.dma_start(b1_sb[:], b1[:])
            b2_sb = consts.tile([P, ND], dt.float32)
            nc.sync.dma_start(b2_sb[:], b2[:])

            for base in range(0, C, SUPER):
                CS = min(SUPER, C - base)
                tiles = _tile_plan(CS)
                nslices = len(tiles)
                offs = [sum(tiles[:i]) for i in range(nslices)]

                x_sb = xp.tile([P, ND, CS], dt.bfloat16, tag="x")
                for d in range(ND):
                    nc.sync.dma_start(x_sb[:, d, :], xt[:, d, base:base + CS])

                g_sb = gp.tile([P, NF, CS], dt.bfloat16, tag="g")
                # layer 1: hT[f,:] = sum_d w1[d,f].T @ xT[d,:]  -> gelu
                # One weight chunk feeds all token slices (LDW amortized),
                # PSUM holds the nslices accumulation banks per f.
                for f in range(NF):
                    w1_sb = w1p.tile([P, ND, P], dt.bfloat16, tag="w1")
                    if base == 0 and f < 2:
                        with tc.high_priority():
                            nc.sync.dma_start(w1_sb[:], w1[f])
                    else:
                        nc.sync.dma_start(w1_sb[:], w1[f])
                    ps = psum.tile([P, nslices, NT], dt.float32, tag="ps")
                    for d in range(ND):
                        for n, (o, nt) in enumerate(zip(offs, tiles)):
                            nc.tensor.matmul(
                                ps[:, n, :nt], w1_sb[:, d, :],
                                x_sb[:, d, o:o + nt],
                                start=(d == 0), stop=(d == ND - 1),
                            )
                    for n, (o, nt) in enumerate(zip(offs, tiles)):
                        nc.scalar.activation(
                            g_sb[:, f, o:o + nt], ps[:, n, :nt], AF.Gelu,
                            bias=b1_sb[:, f:f + 1],
                        )

                # layer 2: yT[dd,:] = sum_ff w2[ff,dd].T @ gT[ff,:]  (+ b2)
                for dd in range(ND):
                    w2_sb = w2p.tile([P, NF, P], dt.bfloat16, tag="w2")
                    with tc.tile_wait_until(0.030 + 0.012 * dd):
                        nc.sync.dma_start(w2_sb[:], w2[dd])
                    ps2 = psum.tile([P, nslices, NT], dt.float32, tag="ps")
                    for ff in range(NF):
                        for n, (o, nt) in enumerate(zip(offs, tiles)):
                            nc.tensor.matmul(
                                ps2[:, n, :nt], w2_sb[:, ff, :],
                                g_sb[:, ff, o:o + nt],
                                start=(ff == 0), stop=(ff == NF - 1),
                            )
                    y_sb = yp.tile([P, CS], dt.float32, tag="y")
                    for n, (o, nt) in enumerate(zip(offs, tiles)):
                        nc.scalar.activation(
                            y_sb[:, o:o + nt], ps2[:, n, :nt], AF.Identity,
                            bias=b2_sb[:, dd:dd + 1],
                        )
                    nc.sync.dma_start(yt[:, dd, base:base + CS], y_sb[:])

    nc.compile()
    return nc


def _build_fp8(C):
    """fp8e4 (e4m3, max +-240) variant with DoubleRow matmuls: the PE packs 2
    fp8 weights per cell, virtualizing the array to 256 contraction rows at 2
    multiplies/cycle -> ~2x bf16/f32r matmul throughput at free dim >=256.
    Weights are pre-scaled by 1024 on host (w1 std 0.02 would land in the
    subnormal range otherwise); the 1/1024 dequant folds into the activation
    scale. x and gelu(h) quantize without scaling (values straddle 1.0).
    fp8 shrinks SBUF so the whole C fits in one pass (no F halving)."""
    import concourse.mybir as mybir
    import concourse.tile as tile
    from concourse import bacc

    dt = mybir.dt
    AF = mybir.ActivationFunctionType
    DR = mybir.MatmulPerfMode.DoubleRow
    WSCALE = 1.0 / 1024.0
    ACT1 = getattr(AF, _ACT_FUNC)  # Gelu; CoreSim lacks Gelu, tests override

    tiles = _slice_plan_256(C)
    nslices = len(tiles)
    offs = [sum(tiles[:i]) for i in range(nslices)]

    nc = bacc.Bacc("TRN2", target_bir_lowering=False, debug=False)
    xt = nc.dram_tensor("xt", (P, ND, C), dt.float8e4, kind="ExternalInput")
    w1 = nc.dram_tensor("w1", (NF, P, ND, P), dt.float8e4, kind="ExternalInput")
    b1 = nc.dram_tensor("b1", (P, NF), dt.float32, kind="ExternalInput")
    w2 = nc.dram_tensor("w2", (ND, P, NF, P), dt.float8e4, kind="ExternalInput")
    b2 = nc.dram_tensor("b2", (P, ND), dt.float32, kind="ExternalInput")
    yt = nc.dram_tensor("yt", (P, ND, C), dt.bfloat16, kind="ExternalOutput")

    with tile.TileContext(nc) as tc:
        with (
            tc.tile_pool(name="consts", bufs=1) as consts,
            tc.tile_pool(name="xp", bufs=1) as xp,
            tc.tile_pool(name="w1p", bufs=3) as w1p,
            tc.tile_pool(name="w2p", bufs=2) as w2p,
            tc.tile_pool(name="gp", bufs=1) as gp,
            tc.tile_pool(name="yp", bufs=2) as yp,
            tc.tile_pool(name="psum", bufs=2 if nslices > 2 else 3,
                         space="PSUM") as psum,
            tc.tile_pool(name="warmp", bufs=1, space="PSUM") as warmp,
        ):
            # Startup critical path: first MM needs w1[0] + x[d0] + x[d1]
            # only. A DMA queue issues one trigger per ~650ns, so spread the
            # head transfers across all three queues (sync/act/gpsimd) and
            # fan the remaining x out round-robin.
            x_sb = xp.tile([P, ND, C], dt.float8e4, tag="x")
            w1_sbs = []
            with tc.high_priority():
                w1_sb0 = w1p.tile([P, ND, P], dt.float8e4, tag="w1")
                nc.sync.dma_start(w1_sb0[:], w1[0])
                nc.scalar.dma_start(x_sb[:, 0, :], xt[:, 0, :])
                nc.gpsimd.dma_start(x_sb[:, 1, :], xt[:, 1, :])
                w1_sbs.append(w1_sb0)
            x_eng = [nc.scalar, nc.gpsimd, nc.sync]
            for d in range(2, ND):
                x_eng[d % 3].dma_start(x_sb[:, d, :], xt[:, d, :])
            b1_sb = consts.tile([P, NF], dt.float32)
            nc.scalar.dma_start(b1_sb[:], b1[:])
            b2_sb = consts.tile([P, ND], dt.float32)
            nc.scalar.dma_start(b2_sb[:], b2[:])

            # Pre-warm the PE while the head DMAs land: the HAM clock gate
            # holds the array at 1.2GHz until ~3.4us of sustained activity.
            # The dummies keep the PE busy from engine-init until the x/w1
            # deps land, so real MMs run at 2.4GHz from the start.
            warm_sb = consts.tile([P, P], dt.float8e4)
            nc.gpsimd.memset(warm_sb[:], 0)
            warm_ps = warmp.tile([P, 64], dt.float32)
            for _ in range(88):
                nc.tensor.matmul(warm_ps[:], warm_sb[:], warm_sb[:, :64])

            g_sb = gp.tile([P, NF, C], dt.float8e4, tag="g")
            # layer 1: hT[f,:] = sum_d w1[d,f].T @ xT[d,:] -> gelu
            # DoubleRow pairs d-chunks: 4 MMs per f instead of 8.
            for f in range(NF):
                if f == 0:
                    w1_sb = w1_sbs[0]
                else:
                    w1_sb = w1p.tile([P, ND, P], dt.float8e4, tag="w1")
                    nc.sync.dma_start(w1_sb[:], w1[f])
                ps = psum.tile([P, nslices, NT], dt.float32, tag="ps")
                for j in range(ND // 2):
                    for n, (o, nt) in enumerate(zip(offs, tiles)):
                        nc.tensor.matmul(
                            ps[:, n, :nt],
                            w1_sb[:, 2 * j:2 * j + 2, :],
                            x_sb[:, 2 * j:2 * j + 2, o:o + nt],
                            start=(j == 0), stop=(j == ND // 2 - 1),
                            perf_mode=DR,
                        )
                for n, (o, nt) in enumerate(zip(offs, tiles)):
                    nc.scalar.activation(
                        g_sb[:, f, o:o + nt], ps[:, n, :nt], ACT1,
                        bias=b1_sb[:, f:f + 1], scale=WSCALE,
                    )

            # layer 2: yT[dd,:] = sum_ff w2[ff,dd].T @ gT[ff,:]  (+ b2)
            for dd in range(ND):
                w2_sb = w2p.tile([P, NF, P], dt.float8e4, tag="w2")
                nc.gpsimd.dma_start(w2_sb[:], w2[dd])
                ps2 = psum.tile([P, nslices, NT], dt.float32, tag="ps")
                for jj in range(NF // 2):
                    for n, (o, nt) in enumerate(zip(offs, tiles)):
                        nc.tensor.matmul(
                            ps2[:, n, :nt],
                            w2_sb[:, 2 * jj:2 * jj + 2, :],
                            g_sb[:, 2 * jj:2 * jj + 2, o:o + nt],
                            start=(jj == 0), stop=(jj == NF // 2 - 1),
                            perf_mode=DR,
                        )
                # dequant stays on ACT only: a DVE/ACT split measured ~1us
                # WORSE - the extra cross-engine edges lengthen the ~115ns/
                # round semaphore-teardown ladder at kernel exit.
                y_sb = yp.tile([P, C], dt.bfloat16, tag="y")
                for n, (o, nt) in enumerate(zip(offs, tiles)):
                    nc.scalar.activation(
                        y_sb[:, o:o + nt], ps2[:, n, :nt], AF.Identity,
                        bias=b2_sb[:, dd:dd + 1], scale=WSCALE,
                    )
                    nc.sync.dma_start(yt[:, dd, o:o + nt], y_sb[:, o:o + nt])

    nc.compile()
    return nc


def _build_f32r(C):
    """float32r variant: fp32 storage, reduced-precision fast matmul.
    F is processed in two halves so g (fp32) fits in SBUF; y accumulates
    across halves in SBUF."""
    import concourse.mybir as mybir
    import concourse.tile as tile
    from concourse import bacc

    dt = mybir.dt
    AF = mybir.ActivationFunctionType
    NFH = NF // 2

    nc = bacc.Bacc("TRN2", target_bir_lowering=False, debug=False)
    xt = nc.dram_tensor("xt", (P, ND, C), dt.float32r, kind="ExternalInput")
    w1 = nc.dram_tensor("w1", (NF, P, ND, P), dt.float32r, kind="ExternalInput")
    b1 = nc.dram_tensor("b1", (P, NF), dt.float32, kind="ExternalInput")
    w2 = nc.dram_tensor("w2", (ND, P, NF, P), dt.float32r, kind="ExternalInput")
    b2 = nc.dram_tensor("b2", (P, ND), dt.float32, kind="ExternalInput")
    yt = nc.dram_tensor("yt", (P, ND, C), dt.float32, kind="ExternalOutput")

    with tile.TileContext(nc) as tc:
        with (
            tc.tile_pool(name="consts", bufs=1) as consts,
            tc.tile_pool(name="xp", bufs=1) as xp,
            tc.tile_pool(name="w1p", bufs=3) as w1p,
            tc.tile_pool(name="w2p", bufs=2) as w2p,
            tc.tile_pool(name="gp", bufs=1) as gp,
            tc.tile_pool(name="yp", bufs=1) as yp,
            tc.tile_pool(name="psum", bufs=2, space="PSUM") as psum,
        ):
            b1_sb = consts.tile([P, NF], dt.float32)
            nc.sync.dma_start(b1_sb[:], b1[:])
            b2_sb = consts.tile([P, ND], dt.float32)
            nc.sync.dma_start(b2_sb[:], b2[:])

            for base in range(0, C, SUPER_F32R):
                CS = min(SUPER_F32R, C - base)
                tiles = _slice_plan_256(CS)
                nslices = len(tiles)
                offs = [sum(tiles[:i]) for i in range(nslices)]

                x_sb = xp.tile([P, ND, CS], dt.float32r, tag="x")
                for d in range(ND):
                    nc.sync.dma_start(x_sb[:, d, :], xt[:, d, base:base + CS])

                y_sb = yp.tile([P, ND, CS], dt.float32, tag="y")

                for half in range(2):
                    g_sb = gp.tile([P, NFH, CS], dt.float32r, tag="g")
                    for fl in range(NFH):
                        f = half * NFH + fl
                        w1_sb = w1p.tile([P, ND, P], dt.float32r, tag="w1")
                        if base == 0 and f < 2:
                            # first weight chunks must beat the bulk x
                            # transfer so the PE can start early
                            with tc.high_priority():
                                nc.sync.dma_start(w1_sb[:], w1[f])
                        else:
                            nc.sync.dma_start(w1_sb[:], w1[f])
                        ps = psum.tile([P, nslices, NT], dt.float32, tag="ps")
                        for d in range(ND):
                            for n, (o, nt) in enumerate(zip(offs, tiles)):
                                nc.tensor.matmul(
                                    ps[:, n, :nt], w1_sb[:, d, :],
                                    x_sb[:, d, o:o + nt],
                                    start=(d == 0), stop=(d == ND - 1),
                                )
                        for n, (o, nt) in enumerate(zip(offs, tiles)):
                            nc.scalar.activation(
                                g_sb[:, fl, o:o + nt], ps[:, n, :nt], AF.Gelu,
                                bias=b1_sb[:, f:f + 1],
                            )

                    for dd in range(ND):
                        w2_sb = w2p.tile([P, NFH, P], dt.float32r, tag="w2")
                        # keep w2 prefetch off the startup critical path: the
                        # head needs x + early w1 chunks first
                        with tc.tile_wait_until(0.040 + 0.056 * half + 0.007 * dd):
                            nc.sync.dma_start(
                                w2_sb[:], w2[dd, :, half * NFH:(half + 1) * NFH, :])
                        ps2 = psum.tile([P, nslices, NT], dt.float32, tag="ps")
                        for fl in range(NFH):
                            for n, (o, nt) in enumerate(zip(offs, tiles)):
                                nc.tensor.matmul(
                                    ps2[:, n, :nt], w2_sb[:, fl, :],
                                    g_sb[:, fl, o:o + nt],
                                    start=(fl == 0), stop=(fl == NFH - 1),
                                )
                        if half == 0:
                            for n, (o, nt) in enumerate(zip(offs, tiles)):
                                nc.scalar.activation(
                                    y_sb[:, dd, o:o + nt], ps2[:, n, :nt],
                                    AF.Identity, bias=b2_sb[:, dd:dd + 1],
                                )
                        else:
                            for n, (o, nt) in enumerate(zip(offs, tiles)):
                                nc.vector.tensor_add(
                                    y_sb[:, dd, o:o + nt],
                                    y_sb[:, dd, o:o + nt], ps2[:, n, :nt],
                                )
                                nc.sync.dma_start(
                                    yt[:, dd, base + o:base + o + nt],
                                    y_sb[:, dd, o:o + nt])

    nc.compile()
    return nc


def _route(x_flat, router_w, router_b):
    """Replicate the reference router on host: softmax -> top-2 -> renorm."""
    logits = (x_flat @ router_w + router_b).astype(np.float64)
    logits -= logits.max(axis=-1, keepdims=True)
    probs = np.exp(logits)
    probs /= probs.sum(axis=-1, keepdims=True)
    # top-k with jax.lax.top_k tie-breaking (lower index wins)
    idx = np.argsort(-probs, axis=-1, kind="stable")[:, :TOPK]
    topw = np.take_along_axis(probs, idx, axis=-1)
    topw = topw / (topw.sum(axis=-1, keepdims=True) + 1e-8)
    return idx.astype(np.int32), topw.astype(np.float32)


PRECISION = "fp8"  # "bf16", "f32r", or "fp8"
FP8 = ml_dtypes.float8_e4m3   # trn2 e4m3: max +-240, matches mybir float8e4
W_SCALE = 1024.0              # host pre-scale for w1/w2 (see _build_fp8)


def _enable_ldw_opt():
    """Rewrite the walrus invocation to enable ldw-opt (elides redundant
    LDWEIGHTS when consecutive matmuls share the stationary operand; our
    f32r pairs issue 3 matmuls per weight chunk)."""
    import concourse.bass_utils as bu
    if getattr(bu, "_ldw_opt_patched", False):
        return
    orig = bu.run_command
    def patched(argv, **kw):
        argv = ["--enable-ldw-opt=true" if a == "--enable-ldw-opt=false" else a
                for a in argv]
        return orig(argv, **kw)
    bu.run_command = patched
    bu._ldw_opt_patched = True


def _ensure_axon_ntff_hook():
    """run_bass_kernel_spmd(trace=True) (also triggered by BASS_TRACE=1)
    imports antenv.axon_hooks, which this image's antenv lacks. Register a
    functional stand-in so tracing works instead of crashing."""
    try:
        import antenv.axon_hooks  # noqa: F401
        return
    except ImportError:
        pass
    try:
        import sys
        import types
        import antenv
        mod = types.ModuleType("antenv.axon_hooks")
        box = [None]
        mod.set_axon_ntff_profile_hook = lambda h: box.__setitem__(0, h)
        mod.get_axon_ntff_profile_hook = lambda: box[0]
        sys.modules["antenv.axon_hooks"] = mod
        antenv.axon_hooks = mod
        try:
            from trn_agent_boot.trn_boot import _ntff_profile_via_ctypes
            mod.set_axon_ntff_profile_hook(
                _ntff_profile_via_ctypes("/opt/axon/libaxon_pjrt.so"))
        except Exception:
            pass
    except Exception:
        pass


def kernel(x, router_w, router_b, w1, b1, w2, b2, _trace=False, _result_box=None):
    if _os.environ.get("LDWOPT") == "1":
        _enable_ldw_opt()
    _ensure_axon_ntff_hook()
    from concourse.bass_utils import run_bass_kernel_spmd

    x = np.asarray(x, dtype=np.float32)
    x_flat = x.reshape(N, D)
    topk_idx, topk_w = _route(x_flat, np.asarray(router_w, np.float32),
                              np.asarray(router_b, np.float32))

    # token lists per expert
    tok_idx = []
    tok_w = []
    for e in range(E):
        t, k = np.nonzero(topk_idx == e)
        tok_idx.append(t.astype(np.int64))
        tok_w.append(topk_w[t, k])
    counts = [len(t) for t in tok_idx]
    cmin = 256 if PRECISION in ("f32r", "fp8") else 128
    C = max(cmin, -(-max(counts) // 32) * 32)
    # Capacity cap: experts above CAP tokens overflow to an exact host-side
    # FFN (a fraction of a percent of the FLOPs). Keeps device slices at a
    # uniform 2x512 and trims the straggler core. 0 disables.
    CAP = int(_os.environ.get("MOE_CAP", "1024"))
    host_jobs = []   # (expert, token_ids, weights)
    if CAP and C > CAP:
        for e in range(E):
            if counts[e] > CAP:
                host_jobs.append((e, tok_idx[e][CAP:], tok_w[e][CAP:]))
                tok_idx[e] = tok_idx[e][:CAP]
                tok_w[e] = tok_w[e][:CAP]
                counts[e] = CAP
        C = CAP

    key = (C, PRECISION)
    if key not in _cache:
        builder = {"f32r": _build_f32r, "fp8": _build_fp8}.get(PRECISION, _build)
        _cache[key] = builder(C)
    nc = _cache[key]

    if PRECISION == "f32r":
        wdt, wscale = np.float32, 1.0
    elif PRECISION == "fp8":
        wdt, wscale = FP8, W_SCALE
    else:
        wdt, wscale = BF16, 1.0
    w1 = np.asarray(w1)
    w2 = np.asarray(w2)
    in_maps = []
    for e in range(E):
        xe = np.zeros((C, D), np.float32)
        xe[:counts[e]] = x_flat[tok_idx[e]]
        xt = np.ascontiguousarray(
            xe.reshape(C, ND, P).transpose(2, 1, 0)).astype(wdt)
        w1e = w1[e] * wscale if wscale != 1.0 else w1[e]
        w2e = w2[e] * wscale if wscale != 1.0 else w2[e]
        w1h = np.ascontiguousarray(
            w1e.reshape(ND, P, NF, P).transpose(2, 1, 0, 3)).astype(wdt)
        w2h = np.ascontiguousarray(
            w2e.reshape(NF, P, ND, P).transpose(2, 1, 0, 3)).astype(wdt)
        b1h = np.ascontiguousarray(
            np.asarray(b1[e], np.float32).reshape(NF, P).T)
        b2h = np.ascontiguousarray(
            np.asarray(b2[e], np.float32).reshape(ND, P).T)
        in_maps.append({"xt": xt, "w1": w1h, "b1": b1h, "w2": w2h, "b2": b2h})

    res = run_bass_kernel_spmd(
        nc, in_maps, core_ids=list(range(E)),
        trace=_trace, trace_cores=list(range(E)) if _trace else None,
        stitch_traces=False,
    )
    if _result_box is not None:
        _result_box.append(res)

    out = x_flat.copy()
    for e in range(E):
        yt = res.results[e]["yt"]                      # [P, ND, C] f32/bf16
        y = yt.transpose(2, 1, 0).reshape(C, D).astype(np.float32)
        cnt = counts[e]
        if cnt:
            out[tok_idx[e]] += tok_w[e][:, None] * y[:cnt]
    for e, toks, tw in host_jobs:   # exact fp32 FFN for capacity overflow
        h = x_flat[toks] @ np.asarray(w1[e], np.float32) + np.asarray(
            b1[e], np.float32)
        g = 0.5 * h * (1.0 + _erf(h / np.sqrt(2.0)))
        y = g @ np.asarray(w2[e], np.float32) + np.asarray(b2[e], np.float32)
        out[toks] += tw[:, None] * y
    return out.reshape(B, S, D)



# revision 46
# speedup vs baseline: 1.0309x; 1.0309x over previous
"""MoE kernel for Trainium2 (8 NeuronCores, expert-parallel sparse dispatch).

Problem (hardcoded): B=2, S=2048, D=1024, E=8 experts, F=4096, top-K=2.
out = x + sum_{k in top2} w_k * (gelu(x @ w1[e_k] + b1[e_k]) @ w2[e_k] + b2[e_k])

Strategy: the router (0.01% of FLOPs) runs on host; tokens are dispatched
expert-parallel to the 8 cores (core i gets expert i's routed tokens) with
capacity 1024 = N*TOPK/E; overflow tokens of hot experts (78 for the seed-0
routing) are computed exactly on host, so every core carries the mean load.
Each core runs a dense FFN over its 1024 tokens in fp8e4 (e4m3) with
DoubleRow matmuls - the PE packs 2 fp8 weights/cell, virtualizing the array
to 256 contraction rows at 2 MACs/cell/cycle, 2x the bf16/f32r rate. All
operands live in transposed layout ([dim, token]); weights are pre-scaled
x1024 on host (else they land in the fp8 subnormal range) and dequantized
for free via the activation-engine scale. x / gelu(h) quantize unscaled.
Head DMAs spread across the sync/act/gpsimd queues (~600ns/trigger,
~90GB/s/queue) so real matmuls start ~12us in; 88 dummy matmuls keep the
PE busy through that window so the HAM clock gate never throttles the real
stream. Host scatter-adds the weighted expert outputs and the residual.

Measured on the seed-0 instance: HW exec 134.3 us (max core, 3-run stable)
vs 285 us for the f32r baseline; rel max err 1.767e-2 (gate 2e-2),
dominated by e4m3 quantization noise (~2.7% per tensor, x/w1/g/w2 equal).
Breakdown: ~7us SPMD init, real MMs 11.8->128us (512 MMs at 213ns = fp8
DoubleRow peak, ~3us idle), ~6.3us tail (final ACT chain + store + drains).
PRECISION selects "fp8" (default), "f32r" (~285 us, 7e-5), or "bf16".
"""

import os as _os

import numpy as np
import ml_dtypes

try:
    from scipy.special import erf as _erf
except ImportError:                     # exact erf fallback, vectorized
    import math as _math
    _erf = np.vectorize(_math.erf, otypes=[np.float64])

B, S, D, E, F, TOPK = 2, 2048, 1024, 8, 4096, 2
N = B * S           # 4096 tokens
P = 128             # partitions
ND = D // P         # 8 chunks of the model dim
NF = F // P         # 32 chunks of the hidden dim
NT = 512            # token tile (matmul free dim; one PSUM bank of fp32)

BF16 = ml_dtypes.bfloat16

_ACT_FUNC = "Gelu"  # overridden to "Identity" by the CoreSim check only

_cache = {}


def _tile_plan(C):
    """Split C tokens into matmul free-dim tiles (multiples of 128)."""
    tiles = [NT] * (C // NT)
    if C % NT:
        tiles.append(C % NT)
    return tiles


# Max tokens processed per weight-stream pass (PSUM: <=4 slices of 512,
# and SBUF must hold x + g for the whole super-tile).
SUPER = 1536
SUPER_F32R = 1152


def _slice_plan_256(C):
    """Split C into slices of multiple-of-32 sizes, each in [256, 512]
    (float32r matmul runs 4x slower below a 256-wide moving operand)."""
    if C <= 512:
        return [C]
    n = -(-C // 512)
    base = C // n // 32 * 32
    sizes = [base] * n
    rem = C - base * n
    i = 0
    while rem > 0:
        add = min(32, rem)
        sizes[i % n] += add
        rem -= add
        i += 1
    return sizes


def _build(C):
    import concourse.mybir as mybir
    import concourse.tile as tile
    from concourse import bacc

    dt = mybir.dt
    AF = mybir.ActivationFunctionType

    nc = bacc.Bacc("TRN2", target_bir_lowering=False, debug=False)
    xt = nc.dram_tensor("xt", (P, ND, C), dt.bfloat16, kind="ExternalInput")
    w1 = nc.dram_tensor("w1", (NF, P, ND, P), dt.bfloat16, kind="ExternalInput")
    b1 = nc.dram_tensor("b1", (P, NF), dt.float32, kind="ExternalInput")
    w2 = nc.dram_tensor("w2", (ND, P, NF, P), dt.bfloat16, kind="ExternalInput")
    b2 = nc.dram_tensor("b2", (P, ND), dt.float32, kind="ExternalInput")
    yt = nc.dram_tensor("yt", (P, ND, C), dt.float32, kind="ExternalOutput")

    with tile.TileContext(nc) as tc:
        with (
            tc.tile_pool(name="consts", bufs=1) as consts,
            tc.tile_pool(name="xp", bufs=1) as xp,
            tc.tile_pool(name="w1p", bufs=4) as w1p,
            tc.tile_pool(name="w2p", bufs=2) as w2p,
            tc.tile_pool(name="gp", bufs=1) as gp,
            tc.tile_pool(name="yp", bufs=2) as yp,
            tc.tile_pool(name="psum", bufs=2, space="PSUM") as psum,
        ):
            b1_sb = consts.tile([P, NF], dt.float32)
            nc.sync.dma_start(b1_sb[:], b1[:])
            b2_sb = consts.tile([P, ND], dt.float32)
            nc.sync.dma_start(b2_sb[:], b2[:])

            for base in range(0, C, SUPER):
                CS = min(SUPER, C - base)
                tiles = _tile_plan(CS)
                nslices = len(tiles)
                offs = [sum(tiles[:i]) for i in range(nslices)]

                x_sb = xp.tile([P, ND, CS], dt.bfloat16, tag="x")
                for d in range(ND):
                    nc.sync.dma_start(x_sb[:, d, :], xt[:, d, base:base + CS])

                g_sb = gp.tile([P, NF, CS], dt.bfloat16, tag="g")
                # layer 1: hT[f,:] = sum_d w1[d,f].T @ xT[d,:]  -> gelu
                # One weight chunk feeds all token slices (LDW amortized),
                # PSUM holds the nslices accumulation banks per f.
                for f in range(NF):
                    w1_sb = w1p.tile([P, ND, P], dt.bfloat16, tag="w1")
                    if base == 0 and f < 2:
                        with tc.high_priority():
                            nc.sync.dma_start(w1_sb[:], w1[f])
                    else:
                        nc.sync.dma_start(w1_sb[:], w1[f])
                    ps = psum.tile([P, nslices, NT], dt.float32, tag="ps")
                    for d in range(ND):
                        for n, (o, nt) in enumerate(zip(offs, tiles)):
                            nc.tensor.matmul(
                                ps[:, n, :nt], w1_sb[:, d, :],
                                x_sb[:, d, o:o + nt],
                                start=(d == 0), stop=(d == ND - 1),
                            )
                    for n, (o, nt) in enumerate(zip(offs, tiles)):
                        nc.scalar.activation(
                            g_sb[:, f, o:o + nt], ps[:, n, :nt], AF.Gelu,
                            bias=b1_sb[:, f:f + 1],
                        )

                # layer 2: yT[dd,:] = sum_ff w2[ff,dd].T @ gT[ff,:]  (+ b2)
                for dd in range(ND):
                    w2_sb = w2p.tile([P, NF, P], dt.bfloat16, tag="w2")
                    with tc.tile_wait_until(0.030 + 0.012 * dd):
                        nc.sync.dma_start(w2_sb[:], w2[dd])
                    ps2 = psum.tile([P, nslices, NT], dt.float32, tag="ps")
                    for ff in range(NF):
                        for n, (o, nt) in enumerate(zip(offs, tiles)):
                            nc.tensor.matmul(
                                ps2[:, n, :nt], w2_sb[:, ff, :],
                                g_sb[:, ff, o:o + nt],
                                start=(ff == 0), stop=(ff == NF - 1),
                            )
                    y_sb = yp.tile([P, CS], dt.float32, tag="y")
                    for n, (o, nt) in enumerate(zip(offs, tiles)):
                        nc.scalar.activation(
                            y_sb[:, o:o + nt], ps2[:, n, :nt], AF.Identity,
                            bias=b2_sb[:, dd:dd + 1],
                        )
                    nc.sync.dma_start(yt[:, dd, base:base + CS], y_sb[:])

    nc.compile()
    return nc


def _build_fp8(C):
    """fp8e4 (e4m3, max +-240) variant with DoubleRow matmuls: the PE packs 2
    fp8 weights per cell, virtualizing the array to 256 contraction rows at 2
    multiplies/cycle -> ~2x bf16/f32r matmul throughput at free dim >=256.
    Weights are pre-scaled by 1024 on host (w1 std 0.02 would land in the
    subnormal range otherwise); the 1/1024 dequant folds into the activation
    scale. x and gelu(h) quantize without scaling (values straddle 1.0).
    fp8 shrinks SBUF so the whole C fits in one pass (no F halving)."""
    import concourse.mybir as mybir
    import concourse.tile as tile
    from concourse import bacc

    dt = mybir.dt
    AF = mybir.ActivationFunctionType
    DR = mybir.MatmulPerfMode.DoubleRow
    WSCALE = 1.0 / 1024.0
    ACT1 = getattr(AF, _ACT_FUNC)  # Gelu; CoreSim lacks Gelu, tests override

    tiles = _slice_plan_256(C)
    nslices = len(tiles)
    offs = [sum(tiles[:i]) for i in range(nslices)]

    nc = bacc.Bacc("TRN2", target_bir_lowering=False, debug=False)
    xt = nc.dram_tensor("xt", (P, ND, C), dt.float8e4, kind="ExternalInput")
    w1 = nc.dram_tensor("w1", (NF, P, ND, P), dt.float8e4, kind="ExternalInput")
    b1 = nc.dram_tensor("b1", (P, NF), dt.float32, kind="ExternalInput")
    w2 = nc.dram_tensor("w2", (ND, P, NF, P), dt.float8e4, kind="ExternalInput")
    b2 = nc.dram_tensor("b2", (P, ND), dt.float32, kind="ExternalInput")
    yt = nc.dram_tensor("yt", (P, ND, C), dt.bfloat16, kind="ExternalOutput")

    with tile.TileContext(nc) as tc:
        with (
            tc.tile_pool(name="consts", bufs=1) as consts,
            tc.tile_pool(name="xp", bufs=1) as xp,
            tc.tile_pool(name="w1p", bufs=4) as w1p,
            tc.tile_pool(name="w2p", bufs=2) as w2p,
            tc.tile_pool(name="gp", bufs=1) as gp,
            tc.tile_pool(name="yp", bufs=2) as yp,
            tc.tile_pool(name="psum", bufs=2 if nslices > 2 else 3,
                         space="PSUM") as psum,
            tc.tile_pool(name="warmp", bufs=1, space="PSUM") as warmp,
        ):
            # Startup critical path: first MM needs w1[0] + x[d0] + x[d1]
            # only. A DMA queue issues one trigger per ~650ns, so spread the
            # head transfers across all three queues (sync/act/gpsimd) and
            # fan the remaining x out round-robin.
            x_sb = xp.tile([P, ND, C], dt.float8e4, tag="x")
            w1_sbs = []
            with tc.high_priority():
                w1_sb0 = w1p.tile([P, ND, P], dt.float8e4, tag="w1")
                nc.sync.dma_start(w1_sb0[:], w1[0])
                nc.scalar.dma_start(x_sb[:, 0, :], xt[:, 0, :])
                nc.gpsimd.dma_start(x_sb[:, 1, :], xt[:, 1, :])
                w1_sbs.append(w1_sb0)
            x_eng = [nc.scalar, nc.gpsimd, nc.sync]
            for d in range(2, ND):
                x_eng[d % 3].dma_start(x_sb[:, d, :], xt[:, d, :])
            b1_sb = consts.tile([P, NF], dt.float32)
            nc.scalar.dma_start(b1_sb[:], b1[:])
            b2_sb = consts.tile([P, ND], dt.float32)
            nc.scalar.dma_start(b2_sb[:], b2[:])

            # Pre-warm the PE while the head DMAs land: the HAM clock gate
            # holds the array at 1.2GHz until ~3.4us of sustained activity.
            # The dummies keep the PE busy from engine-init until the x/w1
            # deps land, so real MMs run at 2.4GHz from the start.
            warm_sb = consts.tile([P, P], dt.float8e4)
            nc.gpsimd.memset(warm_sb[:], 0)
            warm_ps = warmp.tile([P, 64], dt.float32)
            for _ in range(88):
                nc.tensor.matmul(warm_ps[:], warm_sb[:], warm_sb[:, :64])

            g_sb = gp.tile([P, NF, C], dt.float8e4, tag="g")
            # layer 1: hT[f,:] = sum_d w1[d,f].T @ xT[d,:] -> gelu
            # DoubleRow pairs d-chunks: 4 MMs per f instead of 8.
            for f in range(NF):
                if f == 0:
                    w1_sb = w1_sbs[0]
                else:
                    w1_sb = w1p.tile([P, ND, P], dt.float8e4, tag="w1")
                    nc.sync.dma_start(w1_sb[:], w1[f])
                ps = psum.tile([P, nslices, NT], dt.float32, tag="ps")
                for j in range(ND // 2):
                    for n, (o, nt) in enumerate(zip(offs, tiles)):
                        nc.tensor.matmul(
                            ps[:, n, :nt],
                            w1_sb[:, 2 * j:2 * j + 2, :],
                            x_sb[:, 2 * j:2 * j + 2, o:o + nt],
                            start=(j == 0), stop=(j == ND // 2 - 1),
                            perf_mode=DR,
                        )
                for n, (o, nt) in enumerate(zip(offs, tiles)):
                    nc.scalar.activation(
                        g_sb[:, f, o:o + nt], ps[:, n, :nt], ACT1,
                        bias=b1_sb[:, f:f + 1], scale=WSCALE,
                    )

            # layer 2: yT[dd,:] = sum_ff w2[ff,dd].T @ gT[ff,:]  (+ b2)
            for dd in range(ND):
                w2_sb = w2p.tile([P, NF, P], dt.float8e4, tag="w2")
                nc.gpsimd.dma_start(w2_sb[:], w2[dd])
                ps2 = psum.tile([P, nslices, NT], dt.float32, tag="ps")
                for jj in range(NF // 2):
                    for n, (o, nt) in enumerate(zip(offs, tiles)):
                        nc.tensor.matmul(
                            ps2[:, n, :nt],
                            w2_sb[:, 2 * jj:2 * jj + 2, :],
                            g_sb[:, 2 * jj:2 * jj + 2, o:o + nt],
                            start=(jj == 0), stop=(jj == NF // 2 - 1),
                            perf_mode=DR,
                        )
                # dequant stays on ACT only: a DVE/ACT split measured ~1us
                # WORSE - the extra cross-engine edges lengthen the ~115ns/
                # round semaphore-teardown ladder at kernel exit.
                y_sb = yp.tile([P, C], dt.bfloat16, tag="y")
                for n, (o, nt) in enumerate(zip(offs, tiles)):
                    nc.scalar.activation(
                        y_sb[:, o:o + nt], ps2[:, n, :nt], AF.Identity,
                        bias=b2_sb[:, dd:dd + 1], scale=WSCALE,
                    )
                    nc.sync.dma_start(yt[:, dd, o:o + nt], y_sb[:, o:o + nt])

    nc.compile()
    return nc


def _build_f32r(C):
    """float32r variant: fp32 storage, reduced-precision fast matmul.
    F is processed in two halves so g (fp32) fits in SBUF; y accumulates
    across halves in SBUF."""
    import concourse.mybir as mybir
    import concourse.tile as tile
    from concourse import bacc

    dt = mybir.dt
    AF = mybir.ActivationFunctionType
    NFH = NF // 2

    nc = bacc.Bacc("TRN2", target_bir_lowering=False, debug=False)
    xt = nc.dram_tensor("xt", (P, ND, C), dt.float32r, kind="ExternalInput")
    w1 = nc.dram_tensor("w1", (NF, P, ND, P), dt.float32r, kind="ExternalInput")
    b1 = nc.dram_tensor("b1", (P, NF), dt.float32, kind="ExternalInput")
    w2 = nc.dram_tensor("w2", (ND, P, NF, P), dt.float32r, kind="ExternalInput")
    b2 = nc.dram_tensor("b2", (P, ND), dt.float32, kind="ExternalInput")
    yt = nc.dram_tensor("yt", (P, ND, C), dt.float32, kind="ExternalOutput")

    with tile.TileContext(nc) as tc:
        with (
            tc.tile_pool(name="consts", bufs=1) as consts,
            tc.tile_pool(name="xp", bufs=1) as xp,
            tc.tile_pool(name="w1p", bufs=3) as w1p,
            tc.tile_pool(name="w2p", bufs=2) as w2p,
            tc.tile_pool(name="gp", bufs=1) as gp,
            tc.tile_pool(name="yp", bufs=1) as yp,
            tc.tile_pool(name="psum", bufs=2, space="PSUM") as psum,
        ):
            b1_sb = consts.tile([P, NF], dt.float32)
            nc.sync.dma_start(b1_sb[:], b1[:])
            b2_sb = consts.tile([P, ND], dt.float32)
            nc.sync.dma_start(b2_sb[:], b2[:])

            for base in range(0, C, SUPER_F32R):
                CS = min(SUPER_F32R, C - base)
                tiles = _slice_plan_256(CS)
                nslices = len(tiles)
                offs = [sum(tiles[:i]) for i in range(nslices)]

                x_sb = xp.tile([P, ND, CS], dt.float32r, tag="x")
                for d in range(ND):
                    nc.sync.dma_start(x_sb[:, d, :], xt[:, d, base:base + CS])

                y_sb = yp.tile([P, ND, CS], dt.float32, tag="y")

                for half in range(2):
                    g_sb = gp.tile([P, NFH, CS], dt.float32r, tag="g")
                    for fl in range(NFH):
                        f = half * NFH + fl
                        w1_sb = w1p.tile([P, ND, P], dt.float32r, tag="w1")
                        if base == 0 and f < 2:
                            # first weight chunks must beat the bulk x
                            # transfer so the PE can start early
                            with tc.high_priority():
                                nc.sync.dma_start(w1_sb[:], w1[f])
                        else:
                            nc.sync.dma_start(w1_sb[:], w1[f])
                        ps = psum.tile([P, nslices, NT], dt.float32, tag="ps")
                        for d in range(ND):
                            for n, (o, nt) in enumerate(zip(offs, tiles)):
                                nc.tensor.matmul(
                                    ps[:, n, :nt], w1_sb[:, d, :],
                                    x_sb[:, d, o:o + nt],
                                    start=(d == 0), stop=(d == ND - 1),
                                )
                        for n, (o, nt) in enumerate(zip(offs, tiles)):
                            nc.scalar.activation(
                                g_sb[:, fl, o:o + nt], ps[:, n, :nt], AF.Gelu,
                                bias=b1_sb[:, f:f + 1],
                            )

                    for dd in range(ND):
                        w2_sb = w2p.tile([P, NFH, P], dt.float32r, tag="w2")
                        # keep w2 prefetch off the startup critical path: the
                        # head needs x + early w1 chunks first
                        with tc.tile_wait_until(0.040 + 0.056 * half + 0.007 * dd):
                            nc.sync.dma_start(
                                w2_sb[:], w2[dd, :, half * NFH:(half + 1) * NFH, :])
                        ps2 = psum.tile([P, nslices, NT], dt.float32, tag="ps")
                        for fl in range(NFH):
                            for n, (o, nt) in enumerate(zip(offs, tiles)):
                                nc.tensor.matmul(
                                    ps2[:, n, :nt], w2_sb[:, fl, :],
                                    g_sb[:, fl, o:o + nt],
                                    start=(fl == 0), stop=(fl == NFH - 1),
                                )
                        if half == 0:
                            for n, (o, nt) in enumerate(zip(offs, tiles)):
                                nc.scalar.activation(
                                    y_sb[:, dd, o:o + nt], ps2[:, n, :nt],
                                    AF.Identity, bias=b2_sb[:, dd:dd + 1],
                                )
                        else:
                            for n, (o, nt) in enumerate(zip(offs, tiles)):
                                nc.vector.tensor_add(
                                    y_sb[:, dd, o:o + nt],
                                    y_sb[:, dd, o:o + nt], ps2[:, n, :nt],
                                )
                                nc.sync.dma_start(
                                    yt[:, dd, base + o:base + o + nt],
                                    y_sb[:, dd, o:o + nt])

    nc.compile()
    return nc


def _route(x_flat, router_w, router_b):
    """Replicate the reference router on host: softmax -> top-2 -> renorm."""
    logits = (x_flat @ router_w + router_b).astype(np.float64)
    logits -= logits.max(axis=-1, keepdims=True)
    probs = np.exp(logits)
    probs /= probs.sum(axis=-1, keepdims=True)
    # top-k with jax.lax.top_k tie-breaking (lower index wins)
    idx = np.argsort(-probs, axis=-1, kind="stable")[:, :TOPK]
    topw = np.take_along_axis(probs, idx, axis=-1)
    topw = topw / (topw.sum(axis=-1, keepdims=True) + 1e-8)
    return idx.astype(np.int32), topw.astype(np.float32)


PRECISION = "fp8"  # "bf16", "f32r", or "fp8"
FP8 = ml_dtypes.float8_e4m3   # trn2 e4m3: max +-240, matches mybir float8e4
W_SCALE = 1024.0              # host pre-scale for w1/w2 (see _build_fp8)


def _enable_ldw_opt():
    """Rewrite the walrus invocation to enable ldw-opt (elides redundant
    LDWEIGHTS when consecutive matmuls share the stationary operand; our
    f32r pairs issue 3 matmuls per weight chunk)."""
    import concourse.bass_utils as bu
    if getattr(bu, "_ldw_opt_patched", False):
        return
    orig = bu.run_command
    def patched(argv, **kw):
        argv = ["--enable-ldw-opt=true" if a == "--enable-ldw-opt=false" else a
                for a in argv]
        return orig(argv, **kw)
    bu.run_command = patched
    bu._ldw_opt_patched = True


def _ensure_axon_ntff_hook():
    """run_bass_kernel_spmd(trace=True) (also triggered by BASS_TRACE=1)
    imports antenv.axon_hooks, which this image's antenv lacks. Register a
    functional stand-in so tracing works instead of crashing."""
    try:
        import antenv.axon_hooks  # noqa: F401
        return
    except ImportError:
        pass
    try:
        import sys
        import types
        import antenv
        mod = types.ModuleType("antenv.axon_hooks")
        box = [None]
        mod.set_axon_ntff_profile_hook = lambda h: box.__setitem__(0, h)
        mod.get_axon_ntff_profile_hook = lambda: box[0]
        sys.modules["antenv.axon_hooks"] = mod
        antenv.axon_hooks = mod
        try:
            from trn_agent_boot.trn_boot import _ntff_profile_via_ctypes
            mod.set_axon_ntff_profile_hook(
                _ntff_profile_via_ctypes("/opt/axon/libaxon_pjrt.so"))
        except Exception:
            pass
    except Exception:
        pass


def kernel(x, router_w, router_b, w1, b1, w2, b2, _trace=False, _result_box=None):
    if _os.environ.get("LDWOPT") == "1":
        _enable_ldw_opt()
    _ensure_axon_ntff_hook()
    from concourse.bass_utils import run_bass_kernel_spmd

    x = np.asarray(x, dtype=np.float32)
    x_flat = x.reshape(N, D)
    topk_idx, topk_w = _route(x_flat, np.asarray(router_w, np.float32),
                              np.asarray(router_b, np.float32))

    # token lists per expert
    tok_idx = []
    tok_w = []
    for e in range(E):
        t, k = np.nonzero(topk_idx == e)
        tok_idx.append(t.astype(np.int64))
        tok_w.append(topk_w[t, k])
    counts = [len(t) for t in tok_idx]
    cmin = 256 if PRECISION in ("f32r", "fp8") else 128
    C = max(cmin, -(-max(counts) // 32) * 32)
    # Capacity cap: experts above CAP tokens overflow to an exact host-side
    # FFN (a fraction of a percent of the FLOPs). Keeps device slices at a
    # uniform 2x512 and trims the straggler core. 0 disables.
    CAP = int(_os.environ.get("MOE_CAP", "1024"))
    host_jobs = []   # (expert, token_ids, weights)
    if CAP and C > CAP:
        for e in range(E):
            if counts[e] > CAP:
                host_jobs.append((e, tok_idx[e][CAP:], tok_w[e][CAP:]))
                tok_idx[e] = tok_idx[e][:CAP]
                tok_w[e] = tok_w[e][:CAP]
                counts[e] = CAP
        C = CAP

    key = (C, PRECISION)
    if key not in _cache:
        builder = {"f32r": _build_f32r, "fp8": _build_fp8}.get(PRECISION, _build)
        _cache[key] = builder(C)
    nc = _cache[key]

    if PRECISION == "f32r":
        wdt, wscale = np.float32, 1.0
    elif PRECISION == "fp8":
        wdt, wscale = FP8, W_SCALE
    else:
        wdt, wscale = BF16, 1.0
    w1 = np.asarray(w1)
    w2 = np.asarray(w2)
    in_maps = []
    for e in range(E):
        xe = np.zeros((C, D), np.float32)
        xe[:counts[e]] = x_flat[tok_idx[e]]
        xt = np.ascontiguousarray(
            xe.reshape(C, ND, P).transpose(2, 1, 0)).astype(wdt)
        w1e = w1[e] * wscale if wscale != 1.0 else w1[e]
        w2e = w2[e] * wscale if wscale != 1.0 else w2[e]
        w1h = np.ascontiguousarray(
            w1e.reshape(ND, P, NF, P).transpose(2, 1, 0, 3)).astype(wdt)
        w2h = np.ascontiguousarray(
            w2e.reshape(NF, P, ND, P).transpose(2, 1, 0, 3)).astype(wdt)
        b1h = np.ascontiguousarray(
            np.asarray(b1[e], np.float32).reshape(NF, P).T)
        b2h = np.ascontiguousarray(
            np.asarray(b2[e], np.float32).reshape(ND, P).T)
        in_maps.append({"xt": xt, "w1": w1h, "b1": b1h, "w2": w2h, "b2": b2h})

    res = run_bass_kernel_spmd(
        nc, in_maps, core_ids=list(range(E)),
        trace=_trace, trace_cores=list(range(E)) if _trace else None,
        stitch_traces=False,
    )
    if _result_box is not None:
        _result_box.append(res)

    out = x_flat.copy()
    for e in range(E):
        yt = res.results[e]["yt"]                      # [P, ND, C] f32/bf16
        y = yt.transpose(2, 1, 0).reshape(C, D).astype(np.float32)
        cnt = counts[e]
        if cnt:
            out[tok_idx[e]] += tok_w[e][:, None] * y[:cnt]
    for e, toks, tw in host_jobs:   # exact fp32 FFN for capacity overflow
        h = x_flat[toks] @ np.asarray(w1[e], np.float32) + np.asarray(
            b1[e], np.float32)
        g = 0.5 * h * (1.0 + _erf(h / np.sqrt(2.0)))
        y = g @ np.asarray(w2[e], np.float32) + np.asarray(b2[e], np.float32)
        out[toks] += tw[:, None] * y
    return out.reshape(B, S, D)



# revision 47
# speedup vs baseline: 1.0384x; 1.0073x over previous
"""MoE kernel for Trainium2 (8 NeuronCores, expert-parallel sparse dispatch).

Problem (hardcoded): B=2, S=2048, D=1024, E=8 experts, F=4096, top-K=2.
out = x + sum_{k in top2} w_k * (gelu(x @ w1[e_k] + b1[e_k]) @ w2[e_k] + b2[e_k])

Strategy: the router (0.01% of FLOPs) runs on host; tokens are dispatched
expert-parallel to the 8 cores (core i gets expert i's routed tokens) with
capacity 1024 = N*TOPK/E; overflow tokens of hot experts (78 for the seed-0
routing) are computed exactly on host, so every core carries the mean load.
Each core runs a dense FFN over its 1024 tokens in fp8e4 (e4m3) with
DoubleRow matmuls - the PE packs 2 fp8 weights/cell, virtualizing the array
to 256 contraction rows at 2 MACs/cell/cycle, 2x the bf16/f32r rate. All
operands live in transposed layout ([dim, token]); weights are pre-scaled
x1024 on host (else they land in the fp8 subnormal range) and dequantized
for free via the activation-engine scale. x / gelu(h) quantize unscaled.
Head DMAs spread across the sync/act/gpsimd queues (~600ns/trigger,
~90GB/s/queue) so real matmuls start ~12us in; 88 dummy matmuls keep the
PE busy through that window so the HAM clock gate never throttles the real
stream. Host scatter-adds the weighted expert outputs and the residual.

Measured on the seed-0 instance: HW exec 134.3 us (max core, 3-run stable)
vs 285 us for the f32r baseline; rel max err 1.767e-2 (gate 2e-2),
dominated by e4m3 quantization noise (~2.7% per tensor, x/w1/g/w2 equal).
Breakdown: ~7us SPMD init, real MMs 11.8->128us (512 MMs at 213ns = fp8
DoubleRow peak, ~3us idle), ~6.3us tail (final ACT chain + store + drains).
PRECISION selects "fp8" (default), "f32r" (~285 us, 7e-5), or "bf16".
"""

import os as _os

import numpy as np
import ml_dtypes

try:
    from scipy.special import erf as _erf
except ImportError:                     # exact erf fallback, vectorized
    import math as _math
    _erf = np.vectorize(_math.erf, otypes=[np.float64])

B, S, D, E, F, TOPK = 2, 2048, 1024, 8, 4096, 2
N = B * S           # 4096 tokens
P = 128             # partitions
ND = D // P         # 8 chunks of the model dim
NF = F // P         # 32 chunks of the hidden dim
NT = 512            # token tile (matmul free dim; one PSUM bank of fp32)

BF16 = ml_dtypes.bfloat16

_ACT_FUNC = "Gelu"  # overridden to "Identity" by the CoreSim check only

_cache = {}


def _tile_plan(C):
    """Split C tokens into matmul free-dim tiles (multiples of 128)."""
    tiles = [NT] * (C // NT)
    if C % NT:
        tiles.append(C % NT)
    return tiles


# Max tokens processed per weight-stream pass (PSUM: <=4 slices of 512,
# and SBUF must hold x + g for the whole super-tile).
SUPER = 1536
SUPER_F32R = 1152


def _slice_plan_256(C):
    """Split C into slices of multiple-of-32 sizes, each in [256, 512]
    (float32r matmul runs 4x slower below a 256-wide moving operand)."""
    if C <= 512:
        return [C]
    n = -(-C // 512)
    base = C // n // 32 * 32
    sizes = [base] * n
    rem = C - base * n
    i = 0
    while rem > 0:
        add = min(32, rem)
        sizes[i % n] += add
        rem -= add
        i += 1
    return sizes


def _build(C):
    import concourse.mybir as mybir
    import concourse.tile as tile
    from concourse import bacc

    dt = mybir.dt
    AF = mybir.ActivationFunctionType

    nc = bacc.Bacc("TRN2", target_bir_lowering=False, debug=False)
    xt = nc.dram_tensor("xt", (P, ND, C), dt.bfloat16, kind="ExternalInput")
    w1 = nc.dram_tensor("w1", (NF, P, ND, P), dt.bfloat16, kind="ExternalInput")
    b1 = nc.dram_tensor("b1", (P, NF), dt.float32, kind="ExternalInput")
    w2 = nc.dram_tensor("w2", (ND, P, NF, P), dt.bfloat16, kind="ExternalInput")
    b2 = nc.dram_tensor("b2", (P, ND), dt.float32, kind="ExternalInput")
    yt = nc.dram_tensor("yt", (P, ND, C), dt.float32, kind="ExternalOutput")

    with tile.TileContext(nc) as tc:
        with (
            tc.tile_pool(name="consts", bufs=1) as consts,
            tc.tile_pool(name="xp", bufs=1) as xp,
            tc.tile_pool(name="w1p", bufs=4) as w1p,
            tc.tile_pool(name="w2p", bufs=2) as w2p,
            tc.tile_pool(name="gp", bufs=1) as gp,
            tc.tile_pool(name="yp", bufs=2) as yp,
            tc.tile_pool(name="psum", bufs=2, space="PSUM") as psum,
        ):
            b1_sb = consts.tile([P, NF], dt.float32)
            nc.sync.dma_start(b1_sb[:], b1[:])
            b2_sb = consts.tile([P, ND], dt.float32)
            nc.sync.dma_start(b2_sb[:], b2[:])

            for base in range(0, C, SUPER):
                CS = min(SUPER, C - base)
                tiles = _tile_plan(CS)
                nslices = len(tiles)
                offs = [sum(tiles[:i]) for i in range(nslices)]

                x_sb = xp.tile([P, ND, CS], dt.bfloat16, tag="x")
                for d in range(ND):
                    nc.sync.dma_start(x_sb[:, d, :], xt[:, d, base:base + CS])

                g_sb = gp.tile([P, NF, CS], dt.bfloat16, tag="g")
                # layer 1: hT[f,:] = sum_d w1[d,f].T @ xT[d,:]  -> gelu
                # One weight chunk feeds all token slices (LDW amortized),
                # PSUM holds the nslices accumulation banks per f.
                for f in range(NF):
                    w1_sb = w1p.tile([P, ND, P], dt.bfloat16, tag="w1")
                    if base == 0 and f < 2:
                        with tc.high_priority():
                            nc.sync.dma_start(w1_sb[:], w1[f])
                    else:
                        nc.sync.dma_start(w1_sb[:], w1[f])
                    ps = psum.tile([P, nslices, NT], dt.float32, tag="ps")
                    for d in range(ND):
                        for n, (o, nt) in enumerate(zip(offs, tiles)):
                            nc.tensor.matmul(
                                ps[:, n, :nt], w1_sb[:, d, :],
                                x_sb[:, d, o:o + nt],
                                start=(d == 0), stop=(d == ND - 1),
                            )
                    for n, (o, nt) in enumerate(zip(offs, tiles)):
                        nc.scalar.activation(
                            g_sb[:, f, o:o + nt], ps[:, n, :nt], AF.Gelu,
                            bias=b1_sb[:, f:f + 1],
                        )

                # layer 2: yT[dd,:] = sum_ff w2[ff,dd].T @ gT[ff,:]  (+ b2)
                for dd in range(ND):
                    w2_sb = w2p.tile([P, NF, P], dt.bfloat16, tag="w2")
                    with tc.tile_wait_until(0.030 + 0.012 * dd):
                        nc.sync.dma_start(w2_sb[:], w2[dd])
                    ps2 = psum.tile([P, nslices, NT], dt.float32, tag="ps")
                    for ff in range(NF):
                        for n, (o, nt) in enumerate(zip(offs, tiles)):
                            nc.tensor.matmul(
                                ps2[:, n, :nt], w2_sb[:, ff, :],
                                g_sb[:, ff, o:o + nt],
                                start=(ff == 0), stop=(ff == NF - 1),
                            )
                    y_sb = yp.tile([P, CS], dt.float32, tag="y")
                    for n, (o, nt) in enumerate(zip(offs, tiles)):
                        nc.scalar.activation(
                            y_sb[:, o:o + nt], ps2[:, n, :nt], AF.Identity,
                            bias=b2_sb[:, dd:dd + 1],
                        )
                    nc.sync.dma_start(yt[:, dd, base:base + CS], y_sb[:])

    nc.compile()
    return nc


def _build_fp8(C):
    """fp8e4 (e4m3, max +-240) variant with DoubleRow matmuls: the PE packs 2
    fp8 weights per cell, virtualizing the array to 256 contraction rows at 2
    multiplies/cycle -> ~2x bf16/f32r matmul throughput at free dim >=256.
    Weights are pre-scaled by 1024 on host (w1 std 0.02 would land in the
    subnormal range otherwise); the 1/1024 dequant folds into the activation
    scale. x and gelu(h) quantize without scaling (values straddle 1.0).
    fp8 shrinks SBUF so the whole C fits in one pass (no F halving)."""
    import concourse.mybir as mybir
    import concourse.tile as tile
    from concourse import bacc

    dt = mybir.dt
    AF = mybir.ActivationFunctionType
    DR = mybir.MatmulPerfMode.DoubleRow
    WSCALE = 1.0 / 1024.0
    ACT1 = getattr(AF, _ACT_FUNC)  # Gelu; CoreSim lacks Gelu, tests override

    tiles = _slice_plan_256(C)
    nslices = len(tiles)
    offs = [sum(tiles[:i]) for i in range(nslices)]

    nc = bacc.Bacc("TRN2", target_bir_lowering=False, debug=False)
    xt = nc.dram_tensor("xt", (P, ND, C), dt.float8e4, kind="ExternalInput")
    w1 = nc.dram_tensor("w1", (NF, P, ND, P), dt.float8e4, kind="ExternalInput")
    b1 = nc.dram_tensor("b1", (P, NF), dt.float32, kind="ExternalInput")
    w2 = nc.dram_tensor("w2", (ND, P, NF, P), dt.float8e4, kind="ExternalInput")
    b2 = nc.dram_tensor("b2", (P, ND), dt.float32, kind="ExternalInput")
    yt = nc.dram_tensor("yt", (P, ND, C), dt.bfloat16, kind="ExternalOutput")

    with tile.TileContext(nc) as tc:
        with (
            tc.tile_pool(name="consts", bufs=1) as consts,
            tc.tile_pool(name="xp", bufs=1) as xp,
            tc.tile_pool(name="w1p", bufs=4) as w1p,
            tc.tile_pool(name="w2p", bufs=2) as w2p,
            tc.tile_pool(name="gp", bufs=1) as gp,
            tc.tile_pool(name="yp", bufs=2) as yp,
            tc.tile_pool(name="psum", bufs=2 if nslices > 2 else 3,
                         space="PSUM") as psum,
            tc.tile_pool(name="warmp", bufs=1, space="PSUM") as warmp,
        ):
            # Startup critical path: first MM needs w1[0] + x[d0] + x[d1]
            # only. A DMA queue issues one trigger per ~650ns, so spread the
            # head transfers across all three queues (sync/act/gpsimd) and
            # fan the remaining x out round-robin.
            x_sb = xp.tile([P, ND, C], dt.float8e4, tag="x")
            w1_sbs = []
            with tc.high_priority():
                w1_sb0 = w1p.tile([P, ND, P], dt.float8e4, tag="w1")
                nc.sync.dma_start(w1_sb0[:], w1[0])
                nc.scalar.dma_start(x_sb[:, 0, :], xt[:, 0, :])
                nc.gpsimd.dma_start(x_sb[:, 1, :], xt[:, 1, :])
                w1_sbs.append(w1_sb0)
            x_eng = [nc.scalar, nc.gpsimd, nc.sync]
            for d in range(2, ND):
                x_eng[d % 3].dma_start(x_sb[:, d, :], xt[:, d, :])
            b1_sb = consts.tile([P, NF], dt.float32)
            nc.scalar.dma_start(b1_sb[:], b1[:])
            b2_sb = consts.tile([P, ND], dt.float32)
            nc.scalar.dma_start(b2_sb[:], b2[:])

            # Pre-warm the PE while the head DMAs land: the HAM clock gate
            # holds the array at 1.2GHz until ~3.4us of sustained activity.
            # 256-col dummies (~97% duty cycle vs ~50% for 64-col ones, which
            # left the first real MMs cold) keep the PE busy from engine-init
            # until the x/w1 deps land, so real MMs run at 2.4GHz at once.
            warm_sb = consts.tile([P, 2 * P], dt.float8e4)
            nc.gpsimd.memset(warm_sb[:], 0)
            warm_ps = warmp.tile([P, 2 * P], dt.float32)
            for _ in range(32):
                nc.tensor.matmul(warm_ps[:], warm_sb[:, :P], warm_sb[:])

            g_sb = gp.tile([P, NF, C], dt.float8e4, tag="g")
            # layer 1: hT[f,:] = sum_d w1[d,f].T @ xT[d,:] -> gelu
            # DoubleRow pairs d-chunks: 4 MMs per f instead of 8.
            for f in range(NF):
                if f == 0:
                    w1_sb = w1_sbs[0]
                else:
                    w1_sb = w1p.tile([P, ND, P], dt.float8e4, tag="w1")
                    nc.sync.dma_start(w1_sb[:], w1[f])
                ps = psum.tile([P, nslices, NT], dt.float32, tag="ps")
                for j in range(ND // 2):
                    for n, (o, nt) in enumerate(zip(offs, tiles)):
                        nc.tensor.matmul(
                            ps[:, n, :nt],
                            w1_sb[:, 2 * j:2 * j + 2, :],
                            x_sb[:, 2 * j:2 * j + 2, o:o + nt],
                            start=(j == 0), stop=(j == ND // 2 - 1),
                            perf_mode=DR,
                        )
                for n, (o, nt) in enumerate(zip(offs, tiles)):
                    nc.scalar.activation(
                        g_sb[:, f, o:o + nt], ps[:, n, :nt], ACT1,
                        bias=b1_sb[:, f:f + 1], scale=WSCALE,
                    )

            # layer 2: yT[dd,:] = sum_ff w2[ff,dd].T @ gT[ff,:]  (+ b2)
            for dd in range(ND):
                w2_sb = w2p.tile([P, NF, P], dt.float8e4, tag="w2")
                nc.gpsimd.dma_start(w2_sb[:], w2[dd])
                ps2 = psum.tile([P, nslices, NT], dt.float32, tag="ps")
                for jj in range(NF // 2):
                    for n, (o, nt) in enumerate(zip(offs, tiles)):
                        nc.tensor.matmul(
                            ps2[:, n, :nt],
                            w2_sb[:, 2 * jj:2 * jj + 2, :],
                            g_sb[:, 2 * jj:2 * jj + 2, o:o + nt],
                            start=(jj == 0), stop=(jj == NF // 2 - 1),
                            perf_mode=DR,
                        )
                # dequant stays on ACT only: a DVE/ACT split measured ~1us
                # WORSE - the extra cross-engine edges lengthen the ~115ns/
                # round semaphore-teardown ladder at kernel exit.
                y_sb = yp.tile([P, C], dt.bfloat16, tag="y")
                for n, (o, nt) in enumerate(zip(offs, tiles)):
                    nc.scalar.activation(
                        y_sb[:, o:o + nt], ps2[:, n, :nt], AF.Identity,
                        bias=b2_sb[:, dd:dd + 1], scale=WSCALE,
                    )
                    nc.sync.dma_start(yt[:, dd, o:o + nt], y_sb[:, o:o + nt])

    nc.compile()
    return nc


def _build_f32r(C):
    """float32r variant: fp32 storage, reduced-precision fast matmul.
    F is processed in two halves so g (fp32) fits in SBUF; y accumulates
    across halves in SBUF."""
    import concourse.mybir as mybir
    import concourse.tile as tile
    from concourse import bacc

    dt = mybir.dt
    AF = mybir.ActivationFunctionType
    NFH = NF // 2

    nc = bacc.Bacc("TRN2", target_bir_lowering=False, debug=False)
    xt = nc.dram_tensor("xt", (P, ND, C), dt.float32r, kind="ExternalInput")
    w1 = nc.dram_tensor("w1", (NF, P, ND, P), dt.float32r, kind="ExternalInput")
    b1 = nc.dram_tensor("b1", (P, NF), dt.float32, kind="ExternalInput")
    w2 = nc.dram_tensor("w2", (ND, P, NF, P), dt.float32r, kind="ExternalInput")
    b2 = nc.dram_tensor("b2", (P, ND), dt.float32, kind="ExternalInput")
    yt = nc.dram_tensor("yt", (P, ND, C), dt.float32, kind="ExternalOutput")

    with tile.TileContext(nc) as tc:
        with (
            tc.tile_pool(name="consts", bufs=1) as consts,
            tc.tile_pool(name="xp", bufs=1) as xp,
            tc.tile_pool(name="w1p", bufs=3) as w1p,
            tc.tile_pool(name="w2p", bufs=2) as w2p,
            tc.tile_pool(name="gp", bufs=1) as gp,
            tc.tile_pool(name="yp", bufs=1) as yp,
            tc.tile_pool(name="psum", bufs=2, space="PSUM") as psum,
        ):
            b1_sb = consts.tile([P, NF], dt.float32)
            nc.sync.dma_start(b1_sb[:], b1[:])
            b2_sb = consts.tile([P, ND], dt.float32)
            nc.sync.dma_start(b2_sb[:], b2[:])

            for base in range(0, C, SUPER_F32R):
                CS = min(SUPER_F32R, C - base)
                tiles = _slice_plan_256(CS)
                nslices = len(tiles)
                offs = [sum(tiles[:i]) for i in range(nslices)]

                x_sb = xp.tile([P, ND, CS], dt.float32r, tag="x")
                for d in range(ND):
                    nc.sync.dma_start(x_sb[:, d, :], xt[:, d, base:base + CS])

                y_sb = yp.tile([P, ND, CS], dt.float32, tag="y")

                for half in range(2):
                    g_sb = gp.tile([P, NFH, CS], dt.float32r, tag="g")
                    for fl in range(NFH):
                        f = half * NFH + fl
                        w1_sb = w1p.tile([P, ND, P], dt.float32r, tag="w1")
                        if base == 0 and f < 2:
                            # first weight chunks must beat the bulk x
                            # transfer so the PE can start early
                            with tc.high_priority():
                                nc.sync.dma_start(w1_sb[:], w1[f])
                        else:
                            nc.sync.dma_start(w1_sb[:], w1[f])
                        ps = psum.tile([P, nslices, NT], dt.float32, tag="ps")
                        for d in range(ND):
                            for n, (o, nt) in enumerate(zip(offs, tiles)):
                                nc.tensor.matmul(
                                    ps[:, n, :nt], w1_sb[:, d, :],
                                    x_sb[:, d, o:o + nt],
                                    start=(d == 0), stop=(d == ND - 1),
                                )
                        for n, (o, nt) in enumerate(zip(offs, tiles)):
                            nc.scalar.activation(
                                g_sb[:, fl, o:o + nt], ps[:, n, :nt], AF.Gelu,
                                bias=b1_sb[:, f:f + 1],
                            )

                    for dd in range(ND):
                        w2_sb = w2p.tile([P, NFH, P], dt.float32r, tag="w2")
                        # keep w2 prefetch off the startup critical path: the
                        # head needs x + early w1 chunks first
                        with tc.tile_wait_until(0.040 + 0.056 * half + 0.007 * dd):
                            nc.sync.dma_start(
                                w2_sb[:], w2[dd, :, half * NFH:(half + 1) * NFH, :])
                        ps2 = psum.tile([P, nslices, NT], dt.float32, tag="ps")
                        for fl in range(NFH):
                            for n, (o, nt) in enumerate(zip(offs, tiles)):
                                nc.tensor.matmul(
                                    ps2[:, n, :nt], w2_sb[:, fl, :],
                                    g_sb[:, fl, o:o + nt],
                                    start=(fl == 0), stop=(fl == NFH - 1),
                                )
                        if half == 0:
                            for n, (o, nt) in enumerate(zip(offs, tiles)):
                                nc.scalar.activation(
                                    y_sb[:, dd, o:o + nt], ps2[:, n, :nt],
                                    AF.Identity, bias=b2_sb[:, dd:dd + 1],
                                )
                        else:
                            for n, (o, nt) in enumerate(zip(offs, tiles)):
                                nc.vector.tensor_add(
                                    y_sb[:, dd, o:o + nt],
                                    y_sb[:, dd, o:o + nt], ps2[:, n, :nt],
                                )
                                nc.sync.dma_start(
                                    yt[:, dd, base + o:base + o + nt],
                                    y_sb[:, dd, o:o + nt])

    nc.compile()
    return nc


def _route(x_flat, router_w, router_b):
    """Replicate the reference router on host: softmax -> top-2 -> renorm."""
    logits = (x_flat @ router_w + router_b).astype(np.float64)
    logits -= logits.max(axis=-1, keepdims=True)
    probs = np.exp(logits)
    probs /= probs.sum(axis=-1, keepdims=True)
    # top-k with jax.lax.top_k tie-breaking (lower index wins)
    idx = np.argsort(-probs, axis=-1, kind="stable")[:, :TOPK]
    topw = np.take_along_axis(probs, idx, axis=-1)
    topw = topw / (topw.sum(axis=-1, keepdims=True) + 1e-8)
    return idx.astype(np.int32), topw.astype(np.float32)


PRECISION = "fp8"  # "bf16", "f32r", or "fp8"
FP8 = ml_dtypes.float8_e4m3   # trn2 e4m3: max +-240, matches mybir float8e4
W_SCALE = 1024.0              # host pre-scale for w1/w2 (see _build_fp8)


def _enable_ldw_opt():
    """Rewrite the walrus invocation to enable ldw-opt (elides redundant
    LDWEIGHTS when consecutive matmuls share the stationary operand; our
    f32r pairs issue 3 matmuls per weight chunk)."""
    import concourse.bass_utils as bu
    if getattr(bu, "_ldw_opt_patched", False):
        return
    orig = bu.run_command
    def patched(argv, **kw):
        argv = ["--enable-ldw-opt=true" if a == "--enable-ldw-opt=false" else a
                for a in argv]
        return orig(argv, **kw)
    bu.run_command = patched
    bu._ldw_opt_patched = True


def _ensure_axon_ntff_hook():
    """run_bass_kernel_spmd(trace=True) (also triggered by BASS_TRACE=1)
    imports antenv.axon_hooks, which this image's antenv lacks. Register a
    functional stand-in so tracing works instead of crashing."""
    try:
        import antenv.axon_hooks  # noqa: F401
        return
    except ImportError:
        pass
    try:
        import sys
        import types
        import antenv
        mod = types.ModuleType("antenv.axon_hooks")
        box = [None]
        mod.set_axon_ntff_profile_hook = lambda h: box.__setitem__(0, h)
        mod.get_axon_ntff_profile_hook = lambda: box[0]
        sys.modules["antenv.axon_hooks"] = mod
        antenv.axon_hooks = mod
        try:
            from trn_agent_boot.trn_boot import _ntff_profile_via_ctypes
            mod.set_axon_ntff_profile_hook(
                _ntff_profile_via_ctypes("/opt/axon/libaxon_pjrt.so"))
        except Exception:
            pass
    except Exception:
        pass


def kernel(x, router_w, router_b, w1, b1, w2, b2, _trace=False, _result_box=None):
    if _os.environ.get("LDWOPT") == "1":
        _enable_ldw_opt()
    _ensure_axon_ntff_hook()
    from concourse.bass_utils import run_bass_kernel_spmd

    x = np.asarray(x, dtype=np.float32)
    x_flat = x.reshape(N, D)
    topk_idx, topk_w = _route(x_flat, np.asarray(router_w, np.float32),
                              np.asarray(router_b, np.float32))

    # token lists per expert
    tok_idx = []
    tok_w = []
    for e in range(E):
        t, k = np.nonzero(topk_idx == e)
        tok_idx.append(t.astype(np.int64))
        tok_w.append(topk_w[t, k])
    counts = [len(t) for t in tok_idx]
    cmin = 256 if PRECISION in ("f32r", "fp8") else 128
    C = max(cmin, -(-max(counts) // 32) * 32)
    # Capacity cap: experts above CAP tokens overflow to an exact host-side
    # FFN (a fraction of a percent of the FLOPs). Keeps device slices at a
    # uniform 2x512 and trims the straggler core. 0 disables.
    CAP = int(_os.environ.get("MOE_CAP", "1024"))
    host_jobs = []   # (expert, token_ids, weights)
    if CAP and C > CAP:
        for e in range(E):
            if counts[e] > CAP:
                host_jobs.append((e, tok_idx[e][CAP:], tok_w[e][CAP:]))
                tok_idx[e] = tok_idx[e][:CAP]
                tok_w[e] = tok_w[e][:CAP]
                counts[e] = CAP
        C = CAP

    key = (C, PRECISION)
    if key not in _cache:
        builder = {"f32r": _build_f32r, "fp8": _build_fp8}.get(PRECISION, _build)
        _cache[key] = builder(C)
    nc = _cache[key]

    if PRECISION == "f32r":
        wdt, wscale = np.float32, 1.0
    elif PRECISION == "fp8":
        wdt, wscale = FP8, W_SCALE
    else:
        wdt, wscale = BF16, 1.0
    w1 = np.asarray(w1)
    w2 = np.asarray(w2)
    in_maps = []
    for e in range(E):
        xe = np.zeros((C, D), np.float32)
        xe[:counts[e]] = x_flat[tok_idx[e]]
        xt = np.ascontiguousarray(
            xe.reshape(C, ND, P).transpose(2, 1, 0)).astype(wdt)
        w1e = w1[e] * wscale if wscale != 1.0 else w1[e]
        w2e = w2[e] * wscale if wscale != 1.0 else w2[e]
        w1h = np.ascontiguousarray(
            w1e.reshape(ND, P, NF, P).transpose(2, 1, 0, 3)).astype(wdt)
        w2h = np.ascontiguousarray(
            w2e.reshape(NF, P, ND, P).transpose(2, 1, 0, 3)).astype(wdt)
        b1h = np.ascontiguousarray(
            np.asarray(b1[e], np.float32).reshape(NF, P).T)
        b2h = np.ascontiguousarray(
            np.asarray(b2[e], np.float32).reshape(ND, P).T)
        in_maps.append({"xt": xt, "w1": w1h, "b1": b1h, "w2": w2h, "b2": b2h})

    res = run_bass_kernel_spmd(
        nc, in_maps, core_ids=list(range(E)),
        trace=_trace, trace_cores=list(range(E)) if _trace else None,
        stitch_traces=False,
    )
    if _result_box is not None:
        _result_box.append(res)

    out = x_flat.copy()
    for e in range(E):
        yt = res.results[e]["yt"]                      # [P, ND, C] f32/bf16
        y = yt.transpose(2, 1, 0).reshape(C, D).astype(np.float32)
        cnt = counts[e]
        if cnt:
            out[tok_idx[e]] += tok_w[e][:, None] * y[:cnt]
    for e, toks, tw in host_jobs:   # exact fp32 FFN for capacity overflow
        h = x_flat[toks] @ np.asarray(w1[e], np.float32) + np.asarray(
            b1[e], np.float32)
        g = 0.5 * h * (1.0 + _erf(h / np.sqrt(2.0)))
        y = g @ np.asarray(w2[e], np.float32) + np.asarray(b2[e], np.float32)
        out[toks] += tw[:, None] * y
    return out.reshape(B, S, D)

